# revision 39
# baseline (speedup 1.0000x reference)
"""DOM pooling (segment mean+max over pulses, then linear projection) on 8 trn2 cores.

Strategy (bf16 + engine-balanced reductions):
  Host: bucket DOMs by exact pulse count k ("classes"); deal DOMs of each class
  round-robin across the 8 cores (identical class structure per core, padded
  with zero doms to an even per-core count m). On each core, consecutive doms
  are PAIRED: SBUF partition p = parity*64 + feat, so one 128-partition column
  holds one slot of two doms. Within a chunk of P dom-pairs the slot buffer is
  slot-major: col = s*P + j  (pair j, slot s), all bf16.

  Device (one NEFF, SPMD on 8 cores), per chunk:
    - DMA load (128, P*k) bf16 (16KB/partition contiguous rows)
    - segment-max: contiguous-halves tensor_tensor tree on DVE (bf16 2x mode)
    - segment-sum: either a DVE add-tree, or folded into the projection on the
      PE via per-slot PSUM-accumulating matmuls -- chosen per chunk to balance
      DVE vs PE time
    - projection: 128x128 block-diag matmuls (mean scale 1/k folded into the
      per-class sum weights) accumulated in PSUM
    - ACT adds bias during PSUM->SBUF copy (downcast bf16); DMA out.

  Host: scatter per-core (128, N2) outputs back to the full (num_doms, 64).
"""
import sys

import numpy as np

for _p in ("/opt/trn_rl_repo",):
    if _p not in sys.path:
        sys.path.append(_p)

import ml_dtypes

from concourse import bacc
import concourse.mybir as mybir
import concourse.tile as tile
from concourse.bass_utils import run_bass_kernel_spmd

NCORES = 8
D = 64
FP32 = mybir.dt.float32
BF16 = mybir.dt.bfloat16
CHUNK_COLS = 8192   # max slot cols per chunk (16KB/partition bf16)
PMAX = 512          # max dom-pairs per chunk (one PSUM bank of f32)

# engine cost constants for load balancing (calibrated from HW traces)
DVE_NS = 0.68    # ns per 128-lane col, bf16 tensor_tensor in 2x mode
PE_NS = 0.40     # ns per col, bf16 matmul
LDW_NS = 50.0    # per-matmul weight load

last_exec_ns = None  # set when KERNEL_TRACE=1


def _f32_to_bf16_u16(a):
    """Round-to-nearest-even f32 -> bf16 bit pattern (uint16)."""
    u = np.ascontiguousarray(a, dtype=np.float32).view(np.uint32)
    return ((u + 0x7FFF + ((u >> 16) & 1)) >> 16).astype(np.uint16)


def _tree_cols(k, P):
    cols = 0
    w = k
    while w > 1:
        h = w // 2
        cols += h * P
        if w & 1:
            cols += P
        w = h
    return cols


def _plan(counts):
    """Shared class/chunk structure (identical on all cores).

    classes: (k, n_k, m, scol, ocol) ; m per-core doms (even, >= ceil(n_k/8))
    chunks:  (rank, k, c0, P, o0, eng) ; c0 slot-col offset, o0 out-col offset
    """
    kmax = int(counts.max()) if counts.size else 0
    n_k = np.bincount(counts, minlength=kmax + 1)
    classes = []
    scol = 0
    ocol = 0
    for k in range(1, kmax + 1):
        if n_k[k] == 0:
            continue
        m = -(-int(n_k[k]) // NCORES)
        m += m & 1
        classes.append((k, int(n_k[k]), m, scol, ocol))
        scol += (m // 2) * k
        ocol += m // 2
    S, N2 = scol, ocol

    raw = []
    for rank, (k, nk, m, sc, oc) in enumerate(classes):
        P_k = max(1, min(PMAX, CHUNK_COLS // k))
        pairs = m // 2
        j = 0
        while j < pairs:
            P = min(P_k, pairs - j)
            raw.append((rank, k, sc + j * k, P, oc + j))
            j += P
    # emission order: a few small chunks first (fast pipeline fill), then
    # large ones, smallest last (short drain tail). Slot/out offsets are
    # absolute, so processing order is free.
    asc = sorted(range(len(raw)), key=lambda i: raw[i][1] * raw[i][3])
    head, tail, mid = asc[:4], asc[4:12][::-1], asc[12:][::-1]
    order = head + mid + tail
    # the k==1 chunk (DMA -> single matmul, no DVE) makes the shortest
    # possible drain chain -- put it last
    k1 = [i for i in order if raw[i][1] == 1]
    order = [i for i in order if raw[i][1] != 1] + k1
    raw = [raw[i] for i in order]
    # engine assignment (greedy balance) in emission order
    chunks = []
    dve = pe = 0.0
    for rank, k, c0, P, o0 in raw:
        if k == 1:
            eng = "-"
            pe += LDW_NS + P * PE_NS
        else:
            tcost = _tree_cols(k, P) * DVE_NS
            dve += tcost  # max tree always on DVE
            # D: sum tree also on DVE; P: sum via accumulating matmuls on PE
            d_dve, d_pe = tcost, 2 * LDW_NS + 2 * P * PE_NS
            p_dve, p_pe = 0.0, (k + 1) * (LDW_NS + P * PE_NS)
            if max(dve + d_dve, pe + d_pe) <= max(dve + p_dve, pe + p_pe):
                eng = "D"
                dve += d_dve
                pe += d_pe
            else:
                eng = "P"
                pe += p_pe
        chunks.append((rank, k, c0, P, o0, eng))
    return classes, chunks, S, N2


def _build_nc(classes, chunks, S, N2):
    nblk = len(classes) + 1  # per-class sum blocks + shared max block
    jmax = len(classes)

    nc = bacc.Bacc(None)
    slots_t = nc.dram_tensor("slots", [128, S], BF16, kind="ExternalInput")
    # weights pre-transposed on host to the SBUF layout (one clean DMA)
    wts_t = nc.dram_tensor("wts", [128, nblk * 128], BF16, kind="ExternalInput")
    b_t = nc.dram_tensor("b", [128, 1], FP32, kind="ExternalInput")
    out_t = nc.dram_tensor("out", [128, N2], BF16, kind="ExternalOutput")

    ADD = mybir.AluOpType.add
    MAX = mybir.AluOpType.max

    def emit_tree(eng, dst, src, k, P, op):
        """Reduce k slot-major blocks of P cols: result lands in dst[:, :P].

        Level 0 reads src, writes dst (dst may be src for in-place); later
        levels run in-place on dst. Contiguous operands keep DVE 2x mode.
        """
        w = k
        first = True
        while w > 1:
            h = w // 2
            a = dst if not first else src
            eng.tensor_tensor(
                out=dst[:, : h * P], in0=a[:, : h * P],
                in1=a[:, h * P : 2 * h * P], op=op,
            )
            if w & 1:
                eng.tensor_tensor(
                    out=dst[:, (h - 1) * P : h * P],
                    in0=dst[:, (h - 1) * P : h * P],
                    in1=a[:, 2 * h * P : (2 * h + 1) * P], op=op,
                )
            w = h
            first = False

    with tile.TileContext(nc) as tc:
        with (
            tc.tile_pool(name="const", bufs=1) as constp,
            tc.tile_pool(name="inp", bufs=6) as inp,
            tc.tile_pool(name="tmpp", bufs=6) as tmpp,
            tc.tile_pool(name="outp", bufs=6) as outp,
            tc.tile_pool(name="psp", bufs=8, space="PSUM") as psp,
        ):
            # weights/bias on the ACT ring so they overlap the first chunk
            # loads on the SP ring (matmuls are the only consumers)
            wt_sb = constp.tile([128, nblk * 128], BF16)
            nc.scalar.dma_start(wt_sb[:], wts_t[:, :])
            b_sb = constp.tile([128, 1], FP32)
            nc.scalar.dma_start(b_sb[:], b_t[:])

            def flush(st):
                """Emit the DVE-dependent matmuls + ACT + out-DMA of a chunk.

                Deferred one chunk so the PE has the next chunk's independent
                sum matmuls to chew on while the DVE tree finishes (PSUM
                groups interleave across chunks -> skip_group_check).
                """
                rank, k, P, o0, eng, ps, in_t, tmp, opened = st
                if eng != "P":
                    nc.tensor.matmul(
                        ps[:, :P],
                        lhsT=wt_sb[:, rank * 128 : (rank + 1) * 128],
                        rhs=tmp[:, :P] if eng == "D" else in_t[:, :P],
                        start=True, stop=(k == 1),
                        skip_group_check=True,
                    )
                if k != 1:
                    nc.tensor.matmul(
                        ps[:, :P],
                        lhsT=wt_sb[:, jmax * 128 : (jmax + 1) * 128],
                        rhs=tmp[:, :P] if eng == "P" else in_t[:, :P],
                        start=False, stop=True,
                        skip_group_check=True,
                    )
                out_sb = outp.tile([128, PMAX], BF16, tag="out")
                nc.scalar.activation(
                    out_sb[:, :P], ps[:, :P],
                    mybir.ActivationFunctionType.Identity, bias=b_sb[:, :1],
                )
                nc.sync.dma_start(out_t[:, o0 : o0 + P], out_sb[:, :P])

            pending = None
            for rank, k, c0, P, o0, eng in chunks:
                cols = P * k
                in_t = inp.tile([128, CHUNK_COLS], BF16, tag="in")
                # split the load across both HWDGE rings (SP + ACT)
                h2 = (cols // 2) if cols >= 64 else 0
                if h2:
                    nc.sync.dma_start(in_t[:, :h2], slots_t[:, c0 : c0 + h2])
                    nc.scalar.dma_start(
                        in_t[:, h2:cols], slots_t[:, c0 + h2 : c0 + cols]
                    )
                else:
                    nc.sync.dma_start(in_t[:, :cols], slots_t[:, c0 : c0 + cols])

                ps = psp.tile([128, PMAX], FP32, space="PSUM", tag="ps")
                tmp = None
                if k == 1:
                    pass  # single combined matmul, emitted in flush
                elif eng == "P":
                    # max tree on DVE (non-destructive, into tmp); sum on PE
                    # via per-slot PSUM-accumulating matmuls (no DVE dep)
                    tmp = tmpp.tile([128, CHUNK_COLS // 2], BF16, tag="tmp")
                    emit_tree(nc.vector, tmp, in_t, k, P, MAX)
                    for s in range(k):
                        nc.tensor.matmul(
                            ps[:, :P],
                            lhsT=wt_sb[:, rank * 128 : (rank + 1) * 128],
                            rhs=in_t[:, s * P : (s + 1) * P],
                            start=(s == 0), stop=False,
                            skip_group_check=True,
                        )
                else:
                    # both trees on DVE: sum into tmp, max in-place on in_t
                    tmp = tmpp.tile([128, CHUNK_COLS // 2], BF16, tag="tmp")
                    emit_tree(nc.vector, tmp, in_t, k, P, ADD)
                    emit_tree(nc.vector, in_t, in_t, k, P, MAX)
                if pending is not None:
                    flush(pending)
                pending = (rank, k, P, o0, eng, ps, in_t, tmp, eng == "P")
            if pending is not None:
                flush(pending)
    nc.finalize()
    return nc


def kernel(pulse_embeddings, pulse_to_dom_idx, num_doms, proj_w, proj_b):
    global last_exec_ns
    import os

    E = np.ascontiguousarray(np.asarray(pulse_embeddings, dtype=np.float32))
    idx = np.asarray(pulse_to_dom_idx).astype(np.int64)
    nd = int(num_doms)
    W = np.asarray(proj_w, dtype=np.float32)   # (D, 2D)
    b = np.asarray(proj_b, dtype=np.float32)   # (D,)
    NP = E.shape[0]

    counts = np.bincount(idx, minlength=nd)
    classes, chunks, S, N2 = _plan(counts)

    # ---- dom assignment --------------------------------------------------
    dom_order = np.argsort(counts, kind="stable")
    n0 = int((counts == 0).sum())
    dom_class = np.full(nd, -1, np.int32)
    dom_core = np.zeros(nd, np.int8)
    dom_pos = np.zeros(nd, np.int32)
    off = n0
    for rank, (k, nk, m, sc, oc) in enumerate(classes):
        doms = dom_order[off : off + nk]
        off += nk
        ar = np.arange(nk, dtype=np.int64)
        dom_class[doms] = rank
        dom_core[doms] = ar % NCORES
        dom_pos[doms] = ar // NCORES

    # pulses grouped by (core, class, pos); within a dom original order
    dom_key = (
        (dom_core.astype(np.int64) << 40)
        | (dom_class.astype(np.int64) << 20)
        | dom_pos.astype(np.int64)
    )
    perm = np.argsort(dom_key[idx], kind="stable").astype(np.int32)

    # pulse count per (core, class): n_c * k
    core_cls_pulses = np.zeros((NCORES, len(classes)), np.int64)
    for rank, (k, nk, m, sc, oc) in enumerate(classes):
        n_c = nk // NCORES + (np.arange(NCORES) < nk % NCORES)
        core_cls_pulses[:, rank] = n_c * k
    core_off = np.concatenate([[0], np.cumsum(core_cls_pulses.sum(axis=1))])

    # ---- slot buffers ----------------------------------------------------
    Eb = _f32_to_bf16_u16(E)                      # (NP, 64) uint16
    E2b = np.vstack([Eb, np.zeros((1, D), np.uint16)])
    Z = NP

    bufs = []
    for c in range(NCORES):
        blocks = []
        p_off = int(core_off[c])
        for rank, (k, nk, m, sc, oc) in enumerate(classes):
            n_c = nk // NCORES + (1 if c < nk % NCORES else 0)
            R = np.full((m, k), Z, np.int32)
            if n_c:
                R[:n_c] = perm[p_off : p_off + n_c * k].reshape(n_c, k)
                p_off += n_c * k
            R2 = R.reshape(m // 2, 2, k)
            P_k = max(1, min(PMAX, CHUNK_COLS // k))
            j = 0
            while j < m // 2:
                P = min(P_k, m // 2 - j)
                blk = R2[j : j + P]                       # (P, 2, k)
                blocks.append(blk.transpose(1, 2, 0).reshape(2, k * P))
                j += P
        ridx = np.concatenate(blocks, axis=1)             # (2, S)
        g = E2b[ridx]                                     # (2, S, 64) uint16
        buf = np.ascontiguousarray(g.transpose(0, 2, 1)).reshape(128, S)
        bufs.append(buf.view(ml_dtypes.bfloat16))

    # ---- weights / bias --------------------------------------------------
    Wsum = W[:, :D]
    Wmax = W[:, D:]

    def blkdiag(M):
        Z2 = np.zeros((128, 128), np.float32)
        Z2[:D, :D] = M
        Z2[D:, D:] = M
        return Z2

    wblocks = []
    for rank, (k, nk, m, sc, oc) in enumerate(classes):
        if k == 1:
            wblocks.append(blkdiag((Wsum + Wmax).T))
        else:
            wblocks.append(blkdiag(Wsum.T / np.float32(k)))
    wblocks.append(blkdiag(Wmax.T))
    # (nblk*128, 128) -> SBUF layout (128, nblk*128): partition p, col j*128+e
    wcat = np.concatenate(wblocks, axis=0).reshape(-1, 128, 128)
    wcat = np.ascontiguousarray(wcat.transpose(1, 0, 2)).reshape(128, -1)
    wts = _f32_to_bf16_u16(wcat).view(ml_dtypes.bfloat16)
    b128 = np.concatenate([b, b]).reshape(128, 1).astype(np.float32)

    # ---- device ----------------------------------------------------------
    nc = _build_nc(classes, chunks, S, N2)
    in_maps = [{"slots": bufs[c], "wts": wts, "b": b128} for c in range(NCORES)]
    trace = os.environ.get("KERNEL_TRACE", "0") == "1"
    kw_ = {}
    if trace:
        import tempfile
        kw_ = dict(trace=True, tmpdir=tempfile.mkdtemp(prefix="kernel_trace_"))
    res = run_bass_kernel_spmd(nc, in_maps, core_ids=list(range(NCORES)), **kw_)
    last_exec_ns = res.exec_time_ns

    # ---- scatter back ----------------------------------------------------
    outs = np.stack(
        [np.asarray(res.results[c]["out"], dtype=np.float32) for c in range(NCORES)]
    )                                                     # (8, 128, N2)
    outs = outs.reshape(NCORES, 2, D, N2)
    cls_ocol = np.array([oc for (k, nk, m, sc, oc) in classes], np.int64)
    real = counts > 0
    d_core = dom_core[real].astype(np.int64)
    d_ocol = cls_ocol[dom_class[real]] + dom_pos[real] // 2
    d_par = dom_pos[real] % 2
    full = np.empty((nd, D), np.float32)
    full[real] = outs[d_core, d_par, :, d_ocol]
    full[~real] = b
    return full
